# revision 1
# baseline (speedup 1.0000x reference)
"""Causal self-attention (B=4, T=2048, C=1024, H=16) on 8 TRN2 NeuronCores.

Sharding: 8 cores = 4 batches x 2 head-groups (8 heads each). Core c = g*4+b
handles batch b, heads 8g..8g+8 (4 pairs of 2). Inside kernel(): the host
transposes x[b] -> xT [C,T], slices/arranges W_attn columns (Wq pre-scaled by
1/sqrt(D)) and W_proj rows per group, runs one Bass/Tile kernel SPMD on cores
0-7, then sums the two group-partial out^T [C,T] per batch and transposes.

Per-core device pipeline (PE matmuls fp32r; Q/K bf16):
  1. QK^T projection -> Q^T/K^T [128(2 heads), T] per pair
  2. V in natural layout [tk, head, 64] + fused ones column (softmax denom)
  3. per (pair, head), per key-tile jt: scores^T = K_jt @ Q^T over the exact
     causal span (f32 PSUM, 1024-chunks) -> ACT exp -> es (f32r SBUF) ->
     diagonal mask-mul -> PV: [V|1]^T @ es accumulating Y^T+sums [65,T] PSUM
  4. normalize: recip(sums) -> gpsimd partition_broadcast -> DVE mul
  5. output projection: out^T[cout,n] = sum_kp Wp_kp^T @ Y^T_kp -> DMA out
"""
import sys
if '/opt/trn_rl_repo' not in sys.path:
    sys.path.insert(0, '/opt/trn_rl_repo')
import numpy as np
import concourse.bacc as bacc
import concourse.tile as tile
import concourse.mybir as mybir
from concourse import bass_utils

F32 = mybir.dt.float32
F32R = mybir.dt.float32r
BF16 = mybir.dt.bfloat16

N_EMBED = 1024
N_HEAD = 16
D = 64
B_FULL, T_FULL, C_FULL = 4, 2048, 1024
N_GROUPS = 2


def build_kernel(T=T_FULL, C=C_FULL, n_pairs=4, reps=1, n_strip=512, qk_dt=BF16):
    HP = n_pairs * 2
    CIN = HP * D
    n_k = C // 128
    n_jt = T // 128
    n_ts = T // n_strip
    jt_per_strip = n_strip // 128

    nc = bacc.Bacc("TRN2", target_bir_lowering=False, debug=False)
    xt_d = nc.dram_tensor("xt", [C, T], F32R, kind="ExternalInput")
    wqk_d = nc.dram_tensor("wqk", [C, n_pairs * 2 * 128], F32R, kind="ExternalInput")
    wv_d = nc.dram_tensor("wv", [C, n_pairs * 128], F32R, kind="ExternalInput")
    wp_d = nc.dram_tensor("wp", [CIN, C], F32R, kind="ExternalInput")
    mask_d = nc.dram_tensor("mask", [128, 128], F32R, kind="ExternalInput")
    outp_d = nc.dram_tensor("outp", [C, T], F32, kind="ExternalOutput")

    xt_r = xt_d.ap().rearrange("(k p) t -> p k t", p=128)
    wqk_r = wqk_d.ap().rearrange("(k p) m -> p k m", p=128)
    wv_r = wv_d.ap().rearrange("(k p) m -> p k m", p=128)
    wp_r = wp_d.ap().rearrange("(k p) m -> p k m", p=128)

    with tile.TileContext(nc) as tc:
        with tc.tile_pool(name="bigp", bufs=1) as bigp, \
             tc.tile_pool(name="wvp", bufs=1) as wvp, \
             tc.tile_pool(name="xwp", bufs=2) as xwp, \
             tc.tile_pool(name="qkp", bufs=1) as qkp, \
             tc.tile_pool(name="vp", bufs=1) as vp, \
             tc.tile_pool(name="maskp", bufs=1) as maskp, \
             tc.tile_pool(name="esp", bufs=2) as esp, \
             tc.tile_pool(name="normp", bufs=1) as normp, \
             tc.tile_pool(name="osbp", bufs=2) as osbp, \
             tc.tile_pool(name="ps_a", bufs=2, space="PSUM") as ps_a, \
             tc.tile_pool(name="ps_y", bufs=1, space="PSUM") as ps_y:

            def body(_i=None, unroll=1):
                mask_sb = maskp.tile([128, 128], F32R)
                nc.sync.dma_start(out=mask_sb[:], in_=mask_d.ap())
                wqk_sb = bigp.tile([128, n_k, n_pairs * 2 * 128], F32R, tag="big")
                nc.sync.dma_start(out=wqk_sb[:], in_=wqk_r)
                wv_sb = wvp.tile([128, n_k, n_pairs * 128], F32R)
                nc.sync.dma_start(out=wv_sb[:], in_=wv_r)

                qt = [qkp.tile([128, T], qk_dt, tag=f"qt{p}", name=f"qt{p}")
                      for p in range(n_pairs)]
                kt = [qkp.tile([128, T], qk_dt, tag=f"kt{p}", name=f"kt{p}")
                      for p in range(n_pairs)]
                v_aug = vp.tile([128, n_jt, HP, 65], F32R)
                nc.vector.memset(v_aug[:].bitcast(F32), 1.0)

                # ---- phase 1: projections, streamed over token strips ----
                for s in range(n_ts):
                    sl = slice(s * n_strip, (s + 1) * n_strip)
                    xs = xwp.tile([128, n_k, n_strip], F32R, tag="xw")
                    nc.sync.dma_start(out=xs[:], in_=xt_r[:, :, sl])
                    for p in range(n_pairs):
                        for qk in range(2):
                            ps = ps_a.tile([128, n_strip], F32, tag="a")
                            for k in range(n_k):
                                nc.tensor.matmul(
                                    ps[:],
                                    wqk_sb[:, k, (p * 2 + qk) * 128:(p * 2 + qk + 1) * 128],
                                    xs[:, k, :],
                                    start=(k == 0), stop=(k == n_k - 1))
                            dst = (qt if qk == 0 else kt)[p]
                            nc.any.tensor_copy(dst[:, sl], ps[:])
                    for nt in range(jt_per_strip):
                        psv = ps_y.tile([128, n_pairs * 128], F32, tag="y")
                        for k in range(n_k):
                            nc.tensor.matmul(
                                psv[:], xs[:, k, nt * 128:(nt + 1) * 128],
                                wv_sb[:, k, :],
                                start=(k == 0), stop=(k == n_k - 1))
                        jt = s * jt_per_strip + nt
                        nc.any.tensor_copy(
                            v_aug[:, jt, :, 0:64],
                            psv[:].rearrange("q (h d) -> q h d", d=D))

                # ---- phase 2: attention per (pair, head) ----
                ysb = bigp.tile([128, n_pairs, T], F32R, tag="big")
                for p in range(n_pairs):
                    for h in range(2):
                        hh = p * 2 + h
                        hs = slice(h * 64, (h + 1) * 64)
                        y_ps = ps_y.tile([65, T], F32, tag="y")
                        CH = min(2 * n_strip, T)
                        for jt in range(n_jt):
                            s0 = jt // jt_per_strip
                            off = 128 * jt - n_strip * s0
                            lo = 128 * jt
                            es = esp.tile([128, T], F32R, tag="es")
                            # per chunk: scores -> exp -> PV (pipelined)
                            for c in range(lo // CH, T // CH):
                                cw_lo = max(lo, c * CH)
                                scores = ps_a.tile([128, CH], F32, tag="a")
                                s_first = cw_lo // n_strip
                                for s in range(s_first, (c + 1) * CH // n_strip):
                                    a = max(cw_lo, s * n_strip)
                                    n = (s + 1) * n_strip - a
                                    if n < 256 and (s + 1) * n_strip - c * CH >= 256:
                                        a = (s + 1) * n_strip - 256
                                        n = 256
                                    nc.tensor.matmul(
                                        scores[:, a - c * CH:a - c * CH + n],
                                        kt[p][hs, lo:lo + 128],
                                        qt[p][hs, a:a + n],
                                        start=True, stop=True)
                                nc.scalar.activation(
                                    out=es[:, cw_lo:(c + 1) * CH],
                                    in_=scores[:, cw_lo - c * CH:CH],
                                    func=mybir.ActivationFunctionType.Exp)
                                # PV strips of this chunk; masked diagonal strip
                                # last so unmasked strips don't wait on the mask.
                                full_start = (s0 + 1 if cw_lo == lo
                                              else (c * CH) // n_strip)
                                for s in range(full_start, (c + 1) * CH // n_strip):
                                    nc.tensor.matmul(
                                        y_ps[:, s * n_strip:(s + 1) * n_strip],
                                        v_aug[:, jt, hh, :],
                                        es[:, s * n_strip:(s + 1) * n_strip],
                                        start=(jt == 0),
                                        stop=(jt == (s + 1) * jt_per_strip - 1),
                                        skip_group_check=True)
                                if cw_lo == lo:
                                    nc.vector.tensor_mul(
                                        es[:, lo:lo + 128], es[:, lo:lo + 128],
                                        mask_sb[:])
                                    pv_n = min(n_strip - off, T - lo)
                                    nc.tensor.matmul(
                                        y_ps[:, lo:lo + pv_n],
                                        v_aug[:, jt, hh, :], es[:, lo:lo + pv_n],
                                        start=(jt == 0),
                                        stop=(jt == (s0 + 1) * jt_per_strip - 1),
                                        skip_group_check=True)
                        # normalize
                        recip = normp.tile([1, T], F32, tag="recip")
                        nc.vector.reciprocal(recip[:], y_ps[64:65, :])
                        bcast = normp.tile([64, T], F32, tag="bcast")
                        nc.gpsimd.partition_broadcast(bcast[:], recip[:])
                        nc.vector.tensor_mul(
                            ysb[h * 64:(h + 1) * 64, p, :], y_ps[0:64, :], bcast[:])

                # ---- phase 3: output projection ----
                wp_sb = xwp.tile([128, CIN // 128, C], F32R, tag="xw")
                nc.sync.dma_start(out=wp_sb[:], in_=wp_r)
                for m in range(C // 128):
                    for s in range(n_ts):
                        sl = slice(s * n_strip, (s + 1) * n_strip)
                        pso = ps_a.tile([128, n_strip], F32, tag="a")
                        for kp in range(CIN // 128):
                            nc.tensor.matmul(
                                pso[:],
                                wp_sb[:, kp, m * 128:(m + 1) * 128],
                                ysb[:, kp, sl],
                                start=(kp == 0), stop=(kp == CIN // 128 - 1))
                        osb = osbp.tile([128, n_strip], F32, tag="osb")
                        nc.any.tensor_copy(osb[:], pso[:])
                        nc.sync.dma_start(
                            out=outp_d.ap()[m * 128:(m + 1) * 128, sl], in_=osb[:])

            if reps == 1:
                body()
            else:
                with tc.For_i(0, reps, 1) as i:
                    body(i)
    nc.compile()
    return nc


def host_inputs(x, W_attn, W_proj, n_groups=N_GROUPS):
    """Per-core input maps. Core order: g * B + b."""
    B, T, C = x.shape
    hp = N_HEAD // n_groups
    n_pairs = hp // 2
    scale = np.float32(1.0 / np.sqrt(D))
    mask = (np.arange(128)[None, :] >= np.arange(128)[:, None]).astype(np.float32)
    in_maps = []
    for g in range(n_groups):
        qk_cols, v_cols = [], []
        for p in range(n_pairs):
            h0 = g * hp + 2 * p
            h1 = h0 + 1
            qk_cols.append(W_attn[:, h0 * D:(h0 + 1) * D] * scale)
            qk_cols.append(W_attn[:, h1 * D:(h1 + 1) * D] * scale)
            qk_cols.append(W_attn[:, C + h0 * D:C + (h0 + 1) * D])
            qk_cols.append(W_attn[:, C + h1 * D:C + (h1 + 1) * D])
            v_cols.append(W_attn[:, 2 * C + h0 * D:2 * C + (h0 + 1) * D])
            v_cols.append(W_attn[:, 2 * C + h1 * D:2 * C + (h1 + 1) * D])
        wqk = np.ascontiguousarray(np.concatenate(qk_cols, axis=1), dtype=np.float32)
        wv = np.ascontiguousarray(np.concatenate(v_cols, axis=1), dtype=np.float32)
        wp = np.ascontiguousarray(W_proj[g * hp * D:(g + 1) * hp * D], dtype=np.float32)
        for b in range(B):
            xt = np.ascontiguousarray(x[b].T, dtype=np.float32)
            in_maps.append({"xt": xt, "wqk": wqk, "wv": wv, "wp": wp, "mask": mask})
    return in_maps


def host_gather(results, B, T, C, n_groups=N_GROUPS):
    out = np.zeros((B, T, C), dtype=np.float32)
    for g in range(n_groups):
        for b in range(B):
            out[b] += results[g * B + b]["outp"].T
    return out


_NC_CACHE = {}


def kernel(x, W_attn, W_proj):
    x = np.asarray(x, dtype=np.float32)
    W_attn = np.asarray(W_attn, dtype=np.float32)
    W_proj = np.asarray(W_proj, dtype=np.float32)
    B, T, C = x.shape
    if "nc" not in _NC_CACHE:
        _NC_CACHE["nc"] = build_kernel(T=T, C=C)
    nc = _NC_CACHE["nc"]
    in_maps = host_inputs(x, W_attn, W_proj)
    res = bass_utils.run_bass_kernel_spmd(nc, in_maps, core_ids=list(range(8)))
    return host_gather(res.results, B, T, C)



# revision 2
# speedup vs baseline: 1.3533x; 1.3533x over previous
"""Causal self-attention (B=4, T=2048, C=1024, H=16) on 8 TRN2 NeuronCores.

Sharding: 8 cores = 4 batches x 2 head-groups (8 heads each). Core c = g*4+b
handles batch b, heads 8g..8g+8. Host transposes x[b] -> xT [C,T] bf16,
slices/arranges W_attn columns (Wq pre-scaled by 1/sqrt(D)) and W_proj rows
per group (all bf16), runs one Bass/Tile kernel SPMD on cores 0-7, sums the
two group-partial out^T [C,T] per batch on host and transposes.

Per-core device pipeline (all matmuls bf16, f32 PSUM):
  phase 1 (pair-streamed): QK^T projection for pair 0 -> V in natural
    layout [key, head, 64] + fused ones column (softmax denominators)
  phase 2: attention "units" = (pair, head, 512-wide q-tile), two units
    interleaved chunk-by-chunk across 2 PSUM slots. Per chunk (2 key
    tiles): scores^T = K_jt @ Q^T over exact causal span -> one ACT exp
    -> DVE mask-mul on diagonal strips -> PV accumulate y^T+denom
    [65, 512] PSUM. Unit finish: DVE recip -> Pool partition_broadcast
    -> DVE mul -> ysb bf16. Next pair's QK projection groups are emitted
    between rounds so PE stays busy while ACT drains exps.
  phase 3: output projection strips interleaved into the last pair's
    rounds; out^T partial [C,T] bf16 DMA'd out.
"""
import sys
if '/opt/trn_rl_repo' not in sys.path:
    sys.path.insert(0, '/opt/trn_rl_repo')
import numpy as np
import concourse.bacc as bacc
import concourse.tile as tile
import concourse.mybir as mybir
from concourse import bass_utils

F32 = mybir.dt.float32
BF16 = mybir.dt.bfloat16
BF_NP = mybir.dt.np(BF16)

N_EMBED = 1024
N_HEAD = 16
D = 64
B_FULL, T_FULL, C_FULL = 4, 2048, 1024
N_GROUPS = 2
EXPF = mybir.ActivationFunctionType.Exp


def build_kernel(T=T_FULL, C=C_FULL, n_pairs=4, reps=1):
    HP = n_pairs * 2            # heads per core (8)
    n_k = C // 128              # contraction chunks (8)
    n_jt = T // 128             # key tiles (16)
    n_qt = T // 512             # q tiles (4)
    NS = T // 512               # token strips (4)

    nc = bacc.Bacc("TRN2", target_bir_lowering=False, debug=False)
    xt_d = nc.dram_tensor("xt", [C, T], BF16, kind="ExternalInput")
    wqk_d = nc.dram_tensor("wqk", [C, n_pairs * 256], BF16, kind="ExternalInput")
    wv_d = nc.dram_tensor("wv", [C, HP * 64], BF16, kind="ExternalInput")
    wp_d = nc.dram_tensor("wp", [HP * 64, C], BF16, kind="ExternalInput")
    mask_d = nc.dram_tensor("mask", [128, 128], BF16, kind="ExternalInput")
    outp_d = nc.dram_tensor("outp", [C, T], BF16, kind="ExternalOutput")

    xt_r = xt_d.ap().rearrange("(k p) t -> p k t", p=128)
    wqk_r = wqk_d.ap().rearrange("(k p) m -> p k m", p=128)
    wv_r = wv_d.ap().rearrange("(k p) m -> p k m", p=128)
    wp_r = wp_d.ap().rearrange("(k p) m -> p k m", p=128)

    with tile.TileContext(nc) as tc:
        with tc.tile_pool(name="wpool", bufs=1) as wpool, \
             tc.tile_pool(name="xpool", bufs=1) as xpool, \
             tc.tile_pool(name="qkpool", bufs=1) as qkpool, \
             tc.tile_pool(name="vpool", bufs=1) as vpool, \
             tc.tile_pool(name="ypool", bufs=1) as ypool, \
             tc.tile_pool(name="espool", bufs=1) as espool, \
             tc.tile_pool(name="npool", bufs=2) as npool, \
             tc.tile_pool(name="ospool", bufs=2) as ospool, \
             tc.tile_pool(name="ps_a", bufs=2, space="PSUM") as ps_a, \
             tc.tile_pool(name="ps_sc", bufs=1, space="PSUM") as ps_sc, \
             tc.tile_pool(name="ps_y", bufs=1, space="PSUM") as ps_y:

            def body(_i=None):
                mask_sb = wpool.tile([128, 128], BF16, tag="mask", name="mask_sb")
                nc.sync.dma_start(out=mask_sb[:], in_=mask_d.ap())
                wqk_sb = wpool.tile([128, n_k, n_pairs * 256], BF16, tag="wqk",
                                    name="wqk_sb")
                nc.sync.dma_start(out=wqk_sb[:], in_=wqk_r)
                wv_sb = wpool.tile([128, n_k, HP * 64], BF16, tag="wv",
                                   name="wv_sb")
                nc.sync.dma_start(out=wv_sb[:], in_=wv_r)
                xt_sb = xpool.tile([128, n_k, T], BF16, tag="xt", name="xt_sb")
                for s in range(NS):
                    sl = slice(s * 512, (s + 1) * 512)
                    nc.sync.dma_start(out=xt_sb[:, :, sl], in_=xt_r[:, :, sl])
                wp_sb = wpool.tile([128, HP * 64 // 128, C], BF16, tag="wp",
                                   name="wp_sb")
                nc.sync.dma_start(out=wp_sb[:], in_=wp_r)

                qt = [qkpool.tile([128, T], BF16, tag=f"qt{p}", name=f"qt{p}")
                      for p in range(n_pairs)]
                kt = [qkpool.tile([128, T], BF16, tag=f"kt{p}", name=f"kt{p}")
                      for p in range(n_pairs)]
                v_aug = vpool.tile([128, n_jt, HP, 65], BF16, tag="va",
                                   name="v_aug")
                nc.vector.memset(v_aug[:, :, :, 64:65], 1.0)
                ysb = ypool.tile([128, n_pairs, T], BF16, tag="y", name="ysb")

                def qk_group(p, s, qk, eng):
                    sl = slice(s * 512, (s + 1) * 512)
                    ps = ps_a.tile([128, 512], F32, tag="a", name="qk_ps")
                    col = p * 256 + qk * 128
                    for k in range(n_k):
                        nc.tensor.matmul(
                            ps[:], wqk_sb[:, k, col:col + 128],
                            xt_sb[:, k, sl],
                            start=(k == 0), stop=(k == n_k - 1))
                    dst = (qt if qk == 0 else kt)[p]
                    eng.tensor_copy(dst[:, sl], ps[:])

                def v_group(jt, eng):
                    ps = ps_a.tile([128, 512], F32, tag="a", name="v_ps")
                    for k in range(n_k):
                        nc.tensor.matmul(
                            ps[:], xt_sb[:, k, jt * 128:(jt + 1) * 128],
                            wv_sb[:, k, :],
                            start=(k == 0), stop=(k == n_k - 1))
                    eng.tensor_copy(
                        v_aug[:, jt, :, 0:64],
                        ps[:].rearrange("q (h d) -> q h d", d=D))

                def unit(p, h, qi, slot):
                    """Attention unit (pair, head, q-tile); yields per chunk."""
                    n_kt = 4 * qi + 4
                    hh = p * 2 + h
                    hs = slice(h * 64, (h + 1) * 64)
                    qsl = slice(qi * 512, (qi + 1) * 512)
                    y_ps = ps_y.tile([65, 512], F32, tag=f"y{slot}",
                                     name=f"y_ps{slot}")
                    es = espool.tile([128, 1024], BF16, tag=f"es{slot}",
                                     name=f"es{slot}")
                    for c in range(n_kt // 2):
                        jts = (2 * c, 2 * c + 1)
                        sts = [0 if jt < 4 * qi else 128 * (jt - 4 * qi)
                               for jt in jts]
                        sc = ps_sc.tile([128, 1024], F32, tag=f"sc{slot}",
                                        name=f"sc{slot}")
                        for idx, jt in enumerate(jts):
                            st = sts[idx]
                            nc.tensor.matmul(
                                sc[:, idx * 512 + st:(idx + 1) * 512],
                                kt[p][hs, jt * 128:(jt + 1) * 128],
                                qt[p][hs, qi * 512 + st:(qi + 1) * 512],
                                start=True, stop=True)
                        if sts == [0, 0]:
                            nc.scalar.activation(
                                out=es[:, 0:1024], in_=sc[:, 0:1024], func=EXPF)
                        else:
                            for idx, jt in enumerate(jts):
                                st = sts[idx]
                                nc.scalar.activation(
                                    out=es[:, idx * 512 + st:(idx + 1) * 512],
                                    in_=sc[:, idx * 512 + st:(idx + 1) * 512],
                                    func=EXPF)
                        for idx, jt in enumerate(jts):
                            if jt >= 4 * qi:  # diagonal tile: mask strip
                                st = sts[idx]
                                dsl = slice(idx * 512 + st, idx * 512 + st + 128)
                                nc.vector.tensor_mul(es[:, dsl], es[:, dsl],
                                                     mask_sb[:])
                        for idx, jt in enumerate(jts):
                            st = sts[idx]
                            va = v_aug[:, jt, hh, :]
                            first = (jt == 0)
                            if jt < 4 * qi:
                                nc.tensor.matmul(
                                    y_ps[:, 0:512], va,
                                    es[:, idx * 512:(idx + 1) * 512],
                                    start=first, stop=False,
                                    skip_group_check=True)
                            else:
                                b = jt - 4 * qi
                                nc.tensor.matmul(
                                    y_ps[:, st:st + 128], va,
                                    es[:, idx * 512 + st:idx * 512 + st + 128],
                                    start=first, stop=True,
                                    skip_group_check=True)
                                if b < 3:
                                    nc.tensor.matmul(
                                        y_ps[:, st + 128:512], va,
                                        es[:, idx * 512 + st + 128:(idx + 1) * 512],
                                        start=first, stop=False,
                                        skip_group_check=True)
                        yield
                    recip = npool.tile([1, 512], F32, tag=f"r{slot}",
                                       name=f"recip{slot}")
                    nc.vector.reciprocal(recip[:], y_ps[64:65, :])
                    bcast = npool.tile([64, 512], F32, tag=f"b{slot}",
                                       name=f"bcast{slot}")
                    nc.gpsimd.partition_broadcast(bcast[:], recip[:])
                    nc.vector.tensor_mul(ysb[hs, p, qsl], y_ps[0:64, :],
                                         bcast[:])

                def ph3_strip(s, eng):
                    sl = slice(s * 512, (s + 1) * 512)
                    for m in range(C // 128):
                        pso = ps_a.tile([128, 512], F32, tag="a", name="pso")
                        for kp in range(n_pairs):
                            nc.tensor.matmul(
                                pso[:], wp_sb[:, kp, m * 128:(m + 1) * 128],
                                ysb[:, kp, sl],
                                start=(kp == 0), stop=(kp == n_pairs - 1))
                        osb = ospool.tile([128, 512], BF16, tag="osb",
                                          name="osb")
                        eng.tensor_copy(osb[:], pso[:])
                        nc.sync.dma_start(
                            out=outp_d.ap()[m * 128:(m + 1) * 128, sl],
                            in_=osb[:])

                # ---- emission ----
                for s in range(NS):
                    for qk in (0, 1):
                        qk_group(0, s, qk, nc.any)
                for jt in range(n_jt):
                    v_group(jt, nc.any)

                for p in range(n_pairs):
                    nxt = [(s, qk) for s in range(NS) for qk in (0, 1)]
                    for qi in range(n_qt):
                        g0 = unit(p, 0, qi, 0)
                        g1 = unit(p, 1, qi, 1)
                        done0 = done1 = False
                        while not (done0 and done1):
                            if not done0:
                                done0 = next(g0, "end") == "end"
                            if not done1:
                                done1 = next(g1, "end") == "end"
                        if p < n_pairs - 1:
                            for _ in range(2):
                                s, qk = nxt.pop(0)
                                qk_group(p + 1, s, qk, nc.vector)
                        elif qi < n_qt - 1:
                            ph3_strip(qi, nc.vector)
                ph3_strip(n_qt - 1, nc.any)

            if reps == 1:
                body()
            else:
                with tc.For_i(0, reps, 1) as i:
                    body(i)
    nc.compile()
    return nc


def host_inputs(x, W_attn, W_proj, n_groups=N_GROUPS):
    """Per-core input maps (bf16). Core order: g * B + b."""
    B, T, C = x.shape
    hp = N_HEAD // n_groups
    n_pairs = hp // 2
    scale = np.float32(1.0 / np.sqrt(D))
    mask = (np.arange(128)[None, :] >= np.arange(128)[:, None]).astype(BF_NP)
    in_maps = []
    for g in range(n_groups):
        qk_cols, v_cols = [], []
        for p in range(n_pairs):
            h0 = g * hp + 2 * p
            qk_cols.append(W_attn[:, h0 * D:(h0 + 2) * D] * scale)   # Q pair
            qk_cols.append(W_attn[:, C + h0 * D:C + (h0 + 2) * D])   # K pair
            v_cols.append(W_attn[:, 2 * C + h0 * D:2 * C + (h0 + 2) * D])
        wqk = np.ascontiguousarray(
            np.concatenate(qk_cols, axis=1)).astype(BF_NP)
        wv = np.ascontiguousarray(np.concatenate(v_cols, axis=1)).astype(BF_NP)
        wp = np.ascontiguousarray(
            W_proj[g * hp * D:(g + 1) * hp * D]).astype(BF_NP)
        for b in range(B):
            xt = np.ascontiguousarray(x[b].T).astype(BF_NP)
            in_maps.append({"xt": xt, "wqk": wqk, "wv": wv, "wp": wp,
                            "mask": mask})
    return in_maps


def host_gather(results, B, T, C, n_groups=N_GROUPS):
    out = np.zeros((B, T, C), dtype=np.float32)
    for g in range(n_groups):
        for b in range(B):
            out[b] += np.asarray(results[g * B + b]["outp"]).astype(np.float32).T
    return out


_NC_CACHE = {}


def kernel(x, W_attn, W_proj):
    x = np.asarray(x, dtype=np.float32)
    W_attn = np.asarray(W_attn, dtype=np.float32)
    W_proj = np.asarray(W_proj, dtype=np.float32)
    B, T, C = x.shape
    if "nc" not in _NC_CACHE:
        _NC_CACHE["nc"] = build_kernel(T=T, C=C)
    nc = _NC_CACHE["nc"]
    in_maps = host_inputs(x, W_attn, W_proj)
    res = bass_utils.run_bass_kernel_spmd(nc, in_maps, core_ids=list(range(8)))
    return host_gather(res.results, B, T, C)


# revision 4
# speedup vs baseline: 1.3855x; 1.0238x over previous
"""Causal self-attention (B=4, T=2048, C=1024, H=16) on 8 TRN2 NeuronCores.

Sharding: 8 cores = 4 batches x 2 head-groups (8 heads each). Core c = g*4+b
handles batch b, heads 8g..8g+8. Host transposes x[b] -> xT [C,T] bf16,
slices/arranges W_attn columns (Wq pre-scaled by 1/sqrt(D)) and W_proj rows
per group (all bf16), runs one Bass/Tile kernel SPMD on cores 0-7, sums the
two group-partial out^T [C,T] per batch on host and transposes.

Per-core device pipeline (all matmuls bf16, f32 PSUM):
  phase 1 (pair-streamed): QK^T projection for pair 0 -> V in natural
    layout [key, head, 64] + fused ones column (softmax denominators)
  phase 2: attention "units" = (pair, head, 512-wide q-tile), two units
    interleaved chunk-by-chunk across 2 PSUM slots. Per chunk (2 key
    tiles): scores^T = K_jt @ Q^T over exact causal span -> one ACT exp
    -> DVE mask-mul on diagonal strips -> PV accumulate y^T+denom
    [65, 512] PSUM. Unit finish: DVE recip -> Pool partition_broadcast
    -> DVE mul -> ysb bf16. Next pair's QK projection groups are emitted
    between rounds so PE stays busy while ACT drains exps.
  phase 3: output projection strips interleaved into the last pair's
    rounds; out^T partial [C,T] bf16 DMA'd out.
"""
import sys
if '/opt/trn_rl_repo' not in sys.path:
    sys.path.insert(0, '/opt/trn_rl_repo')
import numpy as np
import concourse.bacc as bacc
import concourse.tile as tile
import concourse.mybir as mybir
from concourse import bass_utils

F32 = mybir.dt.float32
BF16 = mybir.dt.bfloat16
BF_NP = mybir.dt.np(BF16)

N_EMBED = 1024
N_HEAD = 16
D = 64
B_FULL, T_FULL, C_FULL = 4, 2048, 1024
N_GROUPS = 2
EXPF = mybir.ActivationFunctionType.Exp


def build_kernel(T=T_FULL, C=C_FULL, n_pairs=4, reps=1):
    HP = n_pairs * 2            # heads per core (8)
    n_k = C // 128              # contraction chunks (8)
    n_jt = T // 128             # key tiles (16)
    n_qt = T // 512             # q tiles (4)
    NS = T // 512               # token strips (4)

    nc = bacc.Bacc("TRN2", target_bir_lowering=False, debug=False)
    xt_d = nc.dram_tensor("xt", [C, T], BF16, kind="ExternalInput")
    wqk_d = nc.dram_tensor("wqk", [C, n_pairs * 256], BF16, kind="ExternalInput")
    wv_d = nc.dram_tensor("wv", [C, HP * 64], BF16, kind="ExternalInput")
    wp_d = nc.dram_tensor("wp", [HP * 64, C], BF16, kind="ExternalInput")
    mask_d = nc.dram_tensor("mask", [128, 128], BF16, kind="ExternalInput")
    outp_d = nc.dram_tensor("outp", [C, T], BF16, kind="ExternalOutput")

    xt_r = xt_d.ap().rearrange("(k p) t -> p k t", p=128)
    wqk_r = wqk_d.ap().rearrange("(k p) m -> p k m", p=128)
    wv_r = wv_d.ap().rearrange("(k p) m -> p k m", p=128)
    wp_r = wp_d.ap().rearrange("(k p) m -> p k m", p=128)

    with tile.TileContext(nc) as tc:
        with tc.tile_pool(name="wpool", bufs=1) as wpool, \
             tc.tile_pool(name="xpool", bufs=1) as xpool, \
             tc.tile_pool(name="qkpool", bufs=1) as qkpool, \
             tc.tile_pool(name="vpool", bufs=1) as vpool, \
             tc.tile_pool(name="ypool", bufs=1) as ypool, \
             tc.tile_pool(name="espool", bufs=1) as espool, \
             tc.tile_pool(name="npool", bufs=2) as npool, \
             tc.tile_pool(name="ospool", bufs=2) as ospool, \
             tc.tile_pool(name="ps_a", bufs=2, space="PSUM") as ps_a, \
             tc.tile_pool(name="ps_sc", bufs=1, space="PSUM") as ps_sc, \
             tc.tile_pool(name="ps_y", bufs=1, space="PSUM") as ps_y:

            def body(_i=None):
                wqk_sb = wpool.tile([128, n_k, n_pairs * 256], BF16, tag="wqk",
                                    name="wqk_sb")
                nc.sync.dma_start(out=wqk_sb[:], in_=wqk_r)
                xt_sb = xpool.tile([128, n_k, T], BF16, tag="xt", name="xt_sb")
                for s in range(NS):
                    sl = slice(s * 512, (s + 1) * 512)
                    nc.sync.dma_start(out=xt_sb[:, :, sl], in_=xt_r[:, :, sl])
                wv_sb = wpool.tile([128, n_k, HP * 64], BF16, tag="wv",
                                   name="wv_sb")
                nc.sync.dma_start(out=wv_sb[:], in_=wv_r)
                mask_sb = wpool.tile([128, 128], BF16, tag="mask", name="mask_sb")
                nc.sync.dma_start(out=mask_sb[:], in_=mask_d.ap())
                wp_sb = wpool.tile([128, HP * 64 // 128, C], BF16, tag="wp",
                                   name="wp_sb")
                nc.sync.dma_start(out=wp_sb[:], in_=wp_r)

                qt = [qkpool.tile([128, T], BF16, tag=f"qt{p}", name=f"qt{p}")
                      for p in range(n_pairs)]
                kt = [qkpool.tile([128, T], BF16, tag=f"kt{p}", name=f"kt{p}")
                      for p in range(n_pairs)]
                # v padded to 128 cols: cols 64+ stay 1.0 (col 64 = softmax
                # denominator row; 65..127 harmless extra sums) so the PV
                # stationary operand is a full 128-col weight (FWL-eligible).
                v_aug = vpool.tile([128, n_jt, HP, 128], BF16, tag="va",
                                   name="v_aug")
                nc.vector.memset(v_aug[:, :, :, 64:128], 1.0)
                ysb = ypool.tile([128, n_pairs, T], BF16, tag="y", name="ysb")

                def qk_group(p, s, qk, eng):
                    sl = slice(s * 512, (s + 1) * 512)
                    ps = ps_a.tile([128, 512], F32, tag="a", name="qk_ps")
                    col = p * 256 + qk * 128
                    for k in range(n_k):
                        nc.tensor.matmul(
                            ps[:], wqk_sb[:, k, col:col + 128],
                            xt_sb[:, k, sl],
                            start=(k == 0), stop=(k == n_k - 1))
                    dst = (qt if qk == 0 else kt)[p]
                    eng.tensor_copy(dst[:, sl], ps[:])

                def v_group(jt, eng):
                    ps = ps_a.tile([128, 512], F32, tag="a", name="v_ps")
                    for k in range(n_k):
                        nc.tensor.matmul(
                            ps[:], xt_sb[:, k, jt * 128:(jt + 1) * 128],
                            wv_sb[:, k, :],
                            start=(k == 0), stop=(k == n_k - 1))
                    eng.tensor_copy(
                        v_aug[:, jt, :, 0:64],
                        ps[:].rearrange("q (h d) -> q h d", d=D))

                def unit(p, h, qi, slot):
                    """Attention unit (pair, head, q-tile); yields per chunk."""
                    n_kt = 4 * qi + 4
                    hh = p * 2 + h
                    hs = slice(h * 64, (h + 1) * 64)
                    qsl = slice(qi * 512, (qi + 1) * 512)
                    y_ps = ps_y.tile([128, 512], F32, tag=f"y{slot}",
                                     name=f"y_ps{slot}")
                    es = espool.tile([128, n_kt * 512], BF16, tag=f"es{slot}",
                                     name=f"es{slot}")
                    for c in range(n_kt // 2):
                        jts = (2 * c, 2 * c + 1)
                        sts = [0 if jt < 4 * qi else 128 * (jt - 4 * qi)
                               for jt in jts]
                        sc = ps_sc.tile([128, 1024], F32, tag=f"sc{slot}",
                                        name=f"sc{slot}")
                        for idx, jt in enumerate(jts):
                            st = sts[idx]
                            nc.tensor.matmul(
                                sc[:, idx * 512 + st:(idx + 1) * 512],
                                kt[p][hs, jt * 128:(jt + 1) * 128],
                                qt[p][hs, qi * 512 + st:(qi + 1) * 512],
                                start=True, stop=True)
                        if sts == [0, 0]:
                            nc.scalar.activation(
                                out=es[:, jts[0] * 512:jts[0] * 512 + 1024],
                                in_=sc[:, 0:1024], func=EXPF)
                        else:
                            for idx, jt in enumerate(jts):
                                st = sts[idx]
                                nc.scalar.activation(
                                    out=es[:, jt * 512 + st:(jt + 1) * 512],
                                    in_=sc[:, idx * 512 + st:(idx + 1) * 512],
                                    func=EXPF)
                        for idx, jt in enumerate(jts):
                            if jt >= 4 * qi:  # diagonal tile: mask strip
                                st = sts[idx]
                                dsl = slice(jt * 512 + st, jt * 512 + st + 128)
                                nc.vector.tensor_mul(es[:, dsl], es[:, dsl],
                                                     mask_sb[:])
                        for idx, jt in enumerate(jts):
                            st = sts[idx]
                            nc.tensor.matmul(
                                y_ps[:, st:512], v_aug[:, jt, hh, :],
                                es[:, jt * 512 + st:(jt + 1) * 512],
                                start=(jt == 0), stop=(jt == n_kt - 1),
                                skip_group_check=True)
                        yield
                    recip = npool.tile([1, 512], F32, tag=f"r{slot}",
                                       name=f"recip{slot}")
                    nc.vector.reciprocal(recip[:], y_ps[64:65, :])
                    bcast = npool.tile([64, 512], F32, tag=f"b{slot}",
                                       name=f"bcast{slot}")
                    nc.gpsimd.partition_broadcast(bcast[:], recip[:])
                    nc.vector.tensor_mul(ysb[hs, p, qsl], y_ps[0:64, :],
                                         bcast[:])

                def ph3_strip(s, eng):
                    sl = slice(s * 512, (s + 1) * 512)
                    for m in range(C // 128):
                        pso = ps_a.tile([128, 512], F32, tag="a", name="pso")
                        for kp in range(n_pairs):
                            nc.tensor.matmul(
                                pso[:], wp_sb[:, kp, m * 128:(m + 1) * 128],
                                ysb[:, kp, sl],
                                start=(kp == 0), stop=(kp == n_pairs - 1))
                        osb = ospool.tile([128, 512], BF16, tag="osb",
                                          name="osb")
                        eng.tensor_copy(osb[:], pso[:])
                        nc.sync.dma_start(
                            out=outp_d.ap()[m * 128:(m + 1) * 128, sl],
                            in_=osb[:])

                # ---- emission ----
                for s in range(NS):
                    for qk in (0, 1):
                        qk_group(0, s, qk, nc.any)
                for jt in range(n_jt):
                    v_group(jt, nc.any)

                for p in range(n_pairs):
                    nxt = [(s, qk) for s in range(NS) for qk in (0, 1)]
                    for qi in range(n_qt):
                        g0 = unit(p, 0, qi, 0)
                        g1 = unit(p, 1, qi, 1)
                        done0 = done1 = False
                        while not (done0 and done1):
                            if not done0:
                                done0 = next(g0, "end") == "end"
                            if not done1:
                                done1 = next(g1, "end") == "end"
                        if p < n_pairs - 1:
                            for _ in range(2):
                                s, qk = nxt.pop(0)
                                qk_group(p + 1, s, qk, nc.vector)
                        elif qi < n_qt - 1:
                            ph3_strip(qi, nc.vector)
                ph3_strip(n_qt - 1, nc.any)

            if reps == 1:
                body()
            else:
                with tc.For_i(0, reps, 1) as i:
                    body(i)
    nc.compile()
    return nc


def host_inputs(x, W_attn, W_proj, n_groups=N_GROUPS):
    """Per-core input maps (bf16). Core order: g * B + b."""
    B, T, C = x.shape
    hp = N_HEAD // n_groups
    n_pairs = hp // 2
    scale = np.float32(1.0 / np.sqrt(D))
    mask = (np.arange(128)[None, :] >= np.arange(128)[:, None]).astype(BF_NP)
    in_maps = []
    for g in range(n_groups):
        qk_cols, v_cols = [], []
        for p in range(n_pairs):
            h0 = g * hp + 2 * p
            qk_cols.append(W_attn[:, h0 * D:(h0 + 2) * D] * scale)   # Q pair
            qk_cols.append(W_attn[:, C + h0 * D:C + (h0 + 2) * D])   # K pair
            v_cols.append(W_attn[:, 2 * C + h0 * D:2 * C + (h0 + 2) * D])
        wqk = np.ascontiguousarray(
            np.concatenate(qk_cols, axis=1)).astype(BF_NP)
        wv = np.ascontiguousarray(np.concatenate(v_cols, axis=1)).astype(BF_NP)
        wp = np.ascontiguousarray(
            W_proj[g * hp * D:(g + 1) * hp * D]).astype(BF_NP)
        for b in range(B):
            xt = np.ascontiguousarray(x[b].T).astype(BF_NP)
            in_maps.append({"xt": xt, "wqk": wqk, "wv": wv, "wp": wp,
                            "mask": mask})
    return in_maps


def host_gather(results, B, T, C, n_groups=N_GROUPS):
    out = np.zeros((B, T, C), dtype=np.float32)
    for g in range(n_groups):
        for b in range(B):
            out[b] += np.asarray(results[g * B + b]["outp"]).astype(np.float32).T
    return out


_NC_CACHE = {}


def kernel(x, W_attn, W_proj):
    x = np.asarray(x, dtype=np.float32)
    W_attn = np.asarray(W_attn, dtype=np.float32)
    W_proj = np.asarray(W_proj, dtype=np.float32)
    B, T, C = x.shape
    if "nc" not in _NC_CACHE:
        _NC_CACHE["nc"] = build_kernel(T=T, C=C)
    nc = _NC_CACHE["nc"]
    in_maps = host_inputs(x, W_attn, W_proj)
    res = bass_utils.run_bass_kernel_spmd(nc, in_maps, core_ids=list(range(8)))
    return host_gather(res.results, B, T, C)


# revision 13
# speedup vs baseline: 1.5254x; 1.1010x over previous
"""Causal self-attention (B=4, T=2048, C=1024, H=16) on 8 TRN2 NeuronCores.

Sharding: 8 cores = 4 batches x 2 head-groups (8 heads each). Core c = g*4+b
handles batch b, heads 8g..8g+8. Host transposes x[b] -> xT [C,T] bf16,
slices/arranges W_attn columns (Wq pre-scaled by 1/sqrt(D)) and W_proj rows
per group (all bf16), runs one Bass/Tile kernel SPMD on cores 0-7, sums the
two group-partial out^T [C,T] per batch on host and transposes.

Per-core device pipeline (all matmuls bf16, f32 PSUM):
  phase 1 (pair-streamed): QK^T projection for pair 0 -> V in natural
    layout [key, head, 64] + fused ones column (softmax denominators)
  phase 2: attention "units" = (pair, head, 512-wide q-tile), two units
    interleaved chunk-by-chunk across 2 PSUM slots. Per chunk (2 key
    tiles): scores^T = K_jt @ Q^T over exact causal span -> one ACT exp
    -> DVE mask-mul on diagonal strips -> PV accumulate y^T+denom
    [65, 512] PSUM. Unit finish: DVE recip -> Pool partition_broadcast
    -> DVE mul -> ysb bf16. Next pair's QK projection groups are emitted
    between rounds so PE stays busy while ACT drains exps.
  phase 3: output projection strips interleaved into the last pair's
    rounds; out^T partial [C,T] bf16 DMA'd out.
"""
import sys
if '/opt/trn_rl_repo' not in sys.path:
    sys.path.insert(0, '/opt/trn_rl_repo')
import numpy as np
import concourse.bacc as bacc
import concourse.tile as tile
import concourse.mybir as mybir
from concourse import bass_utils

F32 = mybir.dt.float32
BF16 = mybir.dt.bfloat16
BF_NP = mybir.dt.np(BF16)

N_EMBED = 1024
N_HEAD = 16
D = 64
B_FULL, T_FULL, C_FULL = 4, 2048, 1024
N_GROUPS = 2
EXPF = mybir.ActivationFunctionType.Exp


def build_kernel(T=T_FULL, C=C_FULL, n_pairs=4, reps=1):
    HP = n_pairs * 2            # heads per core (8)
    n_k = C // 128              # contraction chunks (8)
    n_jt = T // 128             # key tiles (16)
    n_qt = T // 512             # q tiles (4)
    NS = T // 512               # token strips (4)

    nc = bacc.Bacc("TRN2", target_bir_lowering=False, debug=False)
    xt_d = nc.dram_tensor("xt", [C, T], BF16, kind="ExternalInput")
    wqk_d = nc.dram_tensor("wqk", [C, n_pairs * 256], BF16, kind="ExternalInput")
    wv_d = nc.dram_tensor("wv", [C, HP * 64], BF16, kind="ExternalInput")
    wp_d = nc.dram_tensor("wp", [HP * 64, C], BF16, kind="ExternalInput")
    mask_d = nc.dram_tensor("mask", [128, 128], BF16, kind="ExternalInput")
    outp_d = nc.dram_tensor("outp", [C, T], BF16, kind="ExternalOutput")

    xt_r = xt_d.ap().rearrange("(k p) t -> p k t", p=128)
    wqk_r = wqk_d.ap().rearrange("(k p) m -> p k m", p=128)
    wv_r = wv_d.ap().rearrange("(k p) m -> p k m", p=128)
    wp_r = wp_d.ap().rearrange("(k p) m -> p k m", p=128)

    with tile.TileContext(nc) as tc:
        with tc.tile_pool(name="wpool", bufs=1) as wpool, \
             tc.tile_pool(name="xpool", bufs=1) as xpool, \
             tc.tile_pool(name="qkpool", bufs=1) as qkpool, \
             tc.tile_pool(name="vpool", bufs=1) as vpool, \
             tc.tile_pool(name="ypool", bufs=1) as ypool, \
             tc.tile_pool(name="espool", bufs=1) as espool, \
             tc.tile_pool(name="npool", bufs=1) as npool, \
             tc.tile_pool(name="ospool", bufs=2) as ospool, \
             tc.tile_pool(name="ps_a", bufs=2, space="PSUM") as ps_a, \
             tc.tile_pool(name="ps_sc", bufs=1, space="PSUM") as ps_sc, \
             tc.tile_pool(name="ps_y", bufs=1, space="PSUM") as ps_y:

            def body(_i=None):
                wqk_sb = wpool.tile([128, n_k, n_pairs * 256], BF16, tag="wqk",
                                    name="wqk_sb")
                nc.sync.dma_start(out=wqk_sb[:], in_=wqk_r)
                xt_sb = xpool.tile([128, n_k, T], BF16, tag="xt", name="xt_sb")
                for s in range(NS):
                    sl = slice(s * 512, (s + 1) * 512)
                    nc.sync.dma_start(out=xt_sb[:, :, sl], in_=xt_r[:, :, sl])
                wv_sb = wpool.tile([128, n_k, HP * 64], BF16, tag="wv",
                                   name="wv_sb")
                nc.sync.dma_start(out=wv_sb[:], in_=wv_r)
                mask_sb = wpool.tile([128, 128], BF16, tag="mask", name="mask_sb")
                nc.sync.dma_start(out=mask_sb[:], in_=mask_d.ap())
                wp_sb = wpool.tile([128, HP * 64 // 128, C], BF16, tag="wp",
                                   name="wp_sb")
                nc.sync.dma_start(out=wp_sb[:], in_=wp_r)

                qt = [qkpool.tile([128, T], BF16, tag=f"qt{p}", name=f"qt{p}")
                      for p in range(n_pairs)]
                # K tiles zero-padded per head: kz[h][p] has head h's 64 dims
                # live and the other 64 rows zero, so the scores matmul runs
                # at K=128 (3x the K=64 column rate) with the full qt as rhs.
                kz = [[qkpool.tile([128, T], BF16, tag=f"kz{h}{p}",
                                   name=f"kz{h}{p}")
                       for p in range(n_pairs)] for h in range(2)]
                for p in range(n_pairs):
                    nc.vector.memset(kz[0][p][64:128, :], 0.0)
                    nc.vector.memset(kz[1][p][0:64, :], 0.0)
                # v padded to 128 cols: cols 64+ stay 1.0 (col 64 = softmax
                # denominator row; 65..127 harmless extra sums) so the PV
                # stationary operand is a full 128-col weight (FWL-eligible).
                v_aug = vpool.tile([128, n_jt, HP, 128], BF16, tag="va",
                                   name="v_aug")
                nc.vector.memset(v_aug[:, :, :, 64:128], 1.0)
                ysb = ypool.tile([128, n_pairs, T], BF16, tag="y", name="ysb")

                def qk_group(p, s, qk, eng):
                    sl = slice(s * 512, (s + 1) * 512)
                    ps = ps_a.tile([128, 512], F32, tag="a", name="qk_ps")
                    col = p * 256 + qk * 128
                    # sequential 256-wide chains (start=True clears the whole
                    # bank's accumulate flags, so chains must not interleave)
                    for half in range(2):
                        hsl = slice(half * 256, half * 256 + 256)
                        for k in range(n_k):
                            nc.tensor.matmul(
                                ps[:, hsl], wqk_sb[:, k, col:col + 128],
                                xt_sb[:, k, s * 512 + half * 256:
                                      s * 512 + half * 256 + 256],
                                start=(k == 0), stop=(k == n_k - 1),
                                skip_group_check=True)
                    if qk == 0:
                        eng.tensor_copy(qt[p][:, sl], ps[:])
                    else:
                        eng.tensor_copy(kz[0][p][0:64, sl], ps[0:64, :])
                        eng.tensor_copy(kz[1][p][64:128, sl], ps[64:128, :])

                def v_group(jt, eng):
                    ps = ps_a.tile([128, 512], F32, tag="a", name="v_ps")
                    for half in range(2):
                        for k in range(n_k):
                            nc.tensor.matmul(
                                ps[:, half * 256:half * 256 + 256],
                                xt_sb[:, k, jt * 128:(jt + 1) * 128],
                                wv_sb[:, k, half * 256:half * 256 + 256],
                                start=(k == 0), stop=(k == n_k - 1),
                                skip_group_check=True)
                    for half in range(2):
                        eng.tensor_copy(
                            v_aug[:, jt, half * 4:half * 4 + 4, 0:64],
                            ps[:, half * 256:half * 256 + 256]
                            .rearrange("q (h d) -> q h d", d=D))

                def unit(p, h, qi, slot):
                    """Attention unit (pair, head, q-tile); yields per chunk."""
                    n_kt = 4 * qi + 4
                    hh = p * 2 + h
                    hs = slice(h * 64, (h + 1) * 64)
                    qsl = slice(qi * 512, (qi + 1) * 512)
                    y_ps = ps_y.tile([128, 512], F32, tag=f"y{slot}",
                                     name=f"y_ps{slot}")
                    es = espool.tile([128, n_kt * 512], BF16, tag=f"es{slot}",
                                     name=f"es{slot}")
                    for c in range(n_kt // 2):
                        jts = (2 * c, 2 * c + 1)
                        sts = [0 if jt < 4 * qi else 128 * (jt - 4 * qi)
                               for jt in jts]
                        sc = ps_sc.tile([128, 1024], F32, tag=f"sc{slot}",
                                        name=f"sc{slot}")
                        for idx, jt in enumerate(jts):
                            st = sts[idx]
                            for a in range(st, 512, 256):
                                w = min(256, 512 - a)
                                nc.tensor.matmul(
                                    sc[:, idx * 512 + a:idx * 512 + a + w],
                                    kz[h][p][:, jt * 128:(jt + 1) * 128],
                                    qt[p][:, qi * 512 + a:qi * 512 + a + w],
                                    start=True, stop=True,
                                    skip_group_check=True)
                        if sts == [0, 0]:
                            nc.scalar.activation(
                                out=es[:, jts[0] * 512:jts[0] * 512 + 1024],
                                in_=sc[:, 0:1024], func=EXPF)
                        else:
                            for idx, jt in enumerate(jts):
                                st = sts[idx]
                                nc.scalar.activation(
                                    out=es[:, jt * 512 + st:(jt + 1) * 512],
                                    in_=sc[:, idx * 512 + st:(idx + 1) * 512],
                                    func=EXPF)
                        for idx, jt in enumerate(jts):
                            if jt >= 4 * qi:  # diagonal tile: mask strip
                                st = sts[idx]
                                dsl = slice(jt * 512 + st, jt * 512 + st + 128)
                                nc.vector.tensor_mul(es[:, dsl], es[:, dsl],
                                                     mask_sb[:])
                        for idx, jt in enumerate(jts):
                            st = sts[idx]
                            if jt == 0:
                                # single full-width start: one whole-bank
                                # flag-clear, later pieces accumulate
                                nc.tensor.matmul(
                                    y_ps[:, 0:512], v_aug[:, jt, hh, :],
                                    es[:, jt * 512:jt * 512 + 512],
                                    start=True, stop=(jt == n_kt - 1),
                                    skip_group_check=True)
                                continue
                            for a in range(st, 512, 256):
                                w = min(256, 512 - a)
                                nc.tensor.matmul(
                                    y_ps[:, a:a + w], v_aug[:, jt, hh, :],
                                    es[:, jt * 512 + a:jt * 512 + a + w],
                                    start=False, stop=(jt == n_kt - 1),
                                    skip_group_check=True)
                        yield
                    recip = npool.tile([1, 512], F32, tag=f"r{slot}",
                                       name=f"recip{slot}")
                    nc.vector.reciprocal(recip[:], y_ps[64:65, :])
                    bcast = npool.tile([64, 512], F32, tag=f"b{slot}",
                                       name=f"bcast{slot}")
                    nc.gpsimd.partition_broadcast(bcast[:], recip[:])
                    nc.vector.tensor_mul(ysb[hs, p, qsl], y_ps[0:64, :],
                                         bcast[:])

                def ph3_strip(s, eng):
                    sl = slice(s * 512, (s + 1) * 512)
                    for m in range(C // 128):
                        pso = ps_a.tile([128, 512], F32, tag="a", name="pso")
                        for half in range(2):
                            for kp in range(n_pairs):
                                nc.tensor.matmul(
                                    pso[:, half * 256:half * 256 + 256],
                                    wp_sb[:, kp, m * 128:(m + 1) * 128],
                                    ysb[:, kp, s * 512 + half * 256:
                                        s * 512 + half * 256 + 256],
                                    start=(kp == 0), stop=(kp == n_pairs - 1),
                                    skip_group_check=True)
                        osb = ospool.tile([128, 512], BF16, tag="osb",
                                          name="osb")
                        eng.tensor_copy(osb[:], pso[:])
                        nc.sync.dma_start(
                            out=outp_d.ap()[m * 128:(m + 1) * 128, sl],
                            in_=osb[:])

                # ---- emission ----
                for s in range(NS):
                    for qk in (0, 1):
                        qk_group(0, s, qk, nc.any)
                for jt in range(n_jt):
                    v_group(jt, nc.any)

                for p in range(n_pairs):
                    nxt = [(s, qk) for s in range(NS) for qk in (0, 1)]
                    for qi in range(n_qt):
                        g0 = unit(p, 0, qi, 0)
                        g1 = unit(p, 1, qi, 1)
                        done0 = done1 = False
                        while not (done0 and done1):
                            if not done0:
                                done0 = next(g0, "end") == "end"
                            if not done1:
                                done1 = next(g1, "end") == "end"
                        if p < n_pairs - 1:
                            for _ in range(2):
                                s, qk = nxt.pop(0)
                                qk_group(p + 1, s, qk, nc.vector)
                        elif qi < n_qt - 1:
                            ph3_strip(qi, nc.vector)
                ph3_strip(n_qt - 1, nc.any)

            if reps == 1:
                body()
            else:
                with tc.For_i(0, reps, 1) as i:
                    body(i)
    nc.compile()
    return nc


def host_inputs(x, W_attn, W_proj, n_groups=N_GROUPS):
    """Per-core input maps (bf16). Core order: g * B + b."""
    B, T, C = x.shape
    hp = N_HEAD // n_groups
    n_pairs = hp // 2
    scale = np.float32(1.0 / np.sqrt(D))
    mask = (np.arange(128)[None, :] >= np.arange(128)[:, None]).astype(BF_NP)
    in_maps = []
    for g in range(n_groups):
        qk_cols, v_cols = [], []
        for p in range(n_pairs):
            h0 = g * hp + 2 * p
            qk_cols.append(W_attn[:, h0 * D:(h0 + 2) * D] * scale)   # Q pair
            qk_cols.append(W_attn[:, C + h0 * D:C + (h0 + 2) * D])   # K pair
            v_cols.append(W_attn[:, 2 * C + h0 * D:2 * C + (h0 + 2) * D])
        wqk = np.ascontiguousarray(
            np.concatenate(qk_cols, axis=1)).astype(BF_NP)
        wv = np.ascontiguousarray(np.concatenate(v_cols, axis=1)).astype(BF_NP)
        wp = np.ascontiguousarray(
            W_proj[g * hp * D:(g + 1) * hp * D]).astype(BF_NP)
        for b in range(B):
            xt = np.ascontiguousarray(x[b].T).astype(BF_NP)
            in_maps.append({"xt": xt, "wqk": wqk, "wv": wv, "wp": wp,
                            "mask": mask})
    return in_maps


def host_gather(results, B, T, C, n_groups=N_GROUPS):
    out = np.zeros((B, T, C), dtype=np.float32)
    for g in range(n_groups):
        for b in range(B):
            out[b] += np.asarray(results[g * B + b]["outp"]).astype(np.float32).T
    return out


_NC_CACHE = {}


def kernel(x, W_attn, W_proj):
    x = np.asarray(x, dtype=np.float32)
    W_attn = np.asarray(W_attn, dtype=np.float32)
    W_proj = np.asarray(W_proj, dtype=np.float32)
    B, T, C = x.shape
    if "nc" not in _NC_CACHE:
        _NC_CACHE["nc"] = build_kernel(T=T, C=C)
    nc = _NC_CACHE["nc"]
    in_maps = host_inputs(x, W_attn, W_proj)
    res = bass_utils.run_bass_kernel_spmd(nc, in_maps, core_ids=list(range(8)))
    return host_gather(res.results, B, T, C)
